# Initial kernel scaffold
#
"""DeltaSynapse (gnn_message_passing) Trainium2 Bass kernel.

Computes I[b,o] = sum_e signs[e,o]*(W[e,o]*(1-f[e,o]) + Wlong[b,e,o]*f[e,o])
                  * Xpre[b,e,o],
with Xpre[b,e,o] = sum_d delaymap[d,e,o]*Xd[d,b,e]  (one-hot delay gather).

Strategy (8 NeuronCores): shard the postsynaptic axis o into 4 quarters of
512 and the presynaptic axis e into 2 halves of 1024; core (h,q) computes
the partial sum over its e-half for its o-quarter. The two e-half partials
are summed on the host (64KB) and the o-quarters concatenated.

On-device per core:
  - Xd is bit-packed once: packed[e,d] = sum_b 2^b * Xd[d,b,e] (PE transpose
    + weighted free-axis reduce).
  - Per e-tile (128 e's x 512 o's):
      Pi[e,o] = sum_d packed[e,d] * dmap[d,e,o]   (one-hot selection => Pi
      holds all 8 per-batch spike masks as an 8-bit integer, exact in fp16)
      m[b] = (uint8(Pi) >> b) & 1                  (per-batch masks)
      T[b] = (A + C*Wlong[b]) * m[b]               (A = sgn*W*(1-f), C=sgn*f)
      I[b,:] += column-sums of T[b] via PE matmul with a one-hot-column
      stationary matrix (lands each batch on its own PSUM partition).
  All bulk tensors are cast f32->f16 by the DMA engines on load; the
  e-reduction accumulates in fp32 PSUM.
"""
import numpy as np
from contextlib import ExitStack

D, B, N = 8, 8, 2048
NO = 512          # o columns per core
NE = 1024         # e rows per core
ET = NE // 128    # e-tiles per core
N_CORES = 8

_NC = None


def _build():
    from concourse import bacc, tile, mybir, masks
    from concourse.alu_op_type import AluOpType as op

    f32 = mybir.dt.float32
    f16 = mybir.dt.float16
    i16 = mybir.dt.int16

    nc = bacc.Bacc("TRN2", target_bir_lowering=False, debug=False)

    dmap_d = nc.dram_tensor("dmap", (D, NE, NO), f32, kind="ExternalInput")
    xd_d = nc.dram_tensor("xd", (D, B, NE), f32, kind="ExternalInput")
    wl_d = nc.dram_tensor("wl", (B, NE, NO), f32, kind="ExternalInput")
    w_d = nc.dram_tensor("w", (NE, NO), f32, kind="ExternalInput")
    stdp_d = nc.dram_tensor("stdp", (NE, NO), f32, kind="ExternalInput")
    sgn_d = nc.dram_tensor("sgn", (NE, NO), f32, kind="ExternalInput")
    out_d = nc.dram_tensor("iout", (B, NO), f32, kind="ExternalOutput")

    with tile.TileContext(nc) as tc, ExitStack() as ctx:
        cpool = ctx.enter_context(tc.tile_pool(name="const", bufs=1))
        pool = ctx.enter_context(tc.tile_pool(name="work", bufs=2))
        pspool = ctx.enter_context(tc.tile_pool(name="pst", bufs=2, space="PSUM"))
        accpool = ctx.enter_context(tc.tile_pool(name="acc", bufs=1, space="PSUM"))

        # ---- first two tiles' loads first: no slot waits exist yet, so
        # these issue immediately and DMA runs during the constant setup
        pre = {}
        for et in range(2):
            esl = slice(et * 128, (et + 1) * 128)
            dm3 = pool.tile([128, D, NO], f16, name=f"dm3_{et}", tag="dm3")
            nc.gpsimd.dma_start(
                dm3[:], dmap_d[:, esl, :].rearrange("d e o -> e d o"))
            wl3 = pool.tile([128, B, NO], f16, name=f"wl3_{et}", tag="wl3")
            nc.gpsimd.dma_start(
                wl3[:], wl_d[:, esl, :].rearrange("b e o -> e b o"))
            w_t = pool.tile([128, NO], f16, name=f"w_{et}", tag="w_t")
            nc.gpsimd.dma_start(w_t[:], w_d[esl, :])
            stdp_t = pool.tile([128, NO], f16, name=f"st_{et}", tag="stdp_t")
            nc.gpsimd.dma_start(stdp_t[:], stdp_d[esl, :])
            sgn_t = pool.tile([128, NO], f16, name=f"sg_{et}", tag="sgn_t")
            nc.gpsimd.dma_start(sgn_t[:], sgn_d[esl, :])
            pre[et] = (dm3, wl3, w_t, stdp_t, sgn_t)

        # ---- constants -------------------------------------------------
        ident = cpool.tile([D * B, D * B], f32)
        masks.make_identity(nc, ident[:])
        ebs = []
        for b in range(B):
            ebt = cpool.tile([128, B], f16, name=f"eb{b}")
            nc.vector.memset(ebt[:], 0.0)
            nc.vector.memset(ebt[:, b:b + 1], 1.0)
            ebs.append(ebt)
        pw = cpool.tile([128, D, B], f32)
        for b in range(B):
            nc.vector.memset(pw[:, :, b], float(1 << b))
        # stack of 8 identity matrices (f16) for building diag(packed[d])
        ident3 = cpool.tile([128, D, 128], f16)
        for d in range(D):
            masks.make_identity(nc, ident3[:, d, :])

        # ---- pack Xd: packed[e, et, d] = sum_b 2^b * Xd[d, b, e] -------
        xd_nat = cpool.tile([D * B, NE], f32)
        nc.sync.dma_start(xd_nat[:], xd_d[:].flatten_outer_dims())
        packed = cpool.tile([128, ET, D], f32)
        for c in range(ET):
            xdt_ps = pspool.tile([128, D * B], f32, name=f"xdt{c}", tag="xdt")
            nc.tensor.matmul(
                xdt_ps[:], xd_nat[:, c * 128:(c + 1) * 128], ident[:],
                is_transpose=True)
            xw = pool.tile([128, D, B], f32, name=f"xw{c}", tag="xw")
            nc.vector.tensor_tensor(
                xw[:], xdt_ps[:].rearrange("e (d b) -> e d b", d=D), pw[:],
                op=op.mult)
            nc.vector.tensor_reduce(
                packed[:, c, :], xw[:], axis=mybir.AxisListType.X, op=op.add)
        packed16 = cpool.tile([128, ET, D], f16)
        nc.vector.tensor_copy(packed16[:], packed[:])

        acc = accpool.tile([B, NO], f32)

        # ---- main loop over e-tiles ------------------------------------
        for et in range(ET):
            esl = slice(et * 128, (et + 1) * 128)

            if et in pre:
                dm3, wl3, w_t, stdp_t, sgn_t = pre[et]
            else:
                dm3 = pool.tile([128, D, NO], f16, tag="dm3")
                nc.gpsimd.dma_start(
                    dm3[:], dmap_d[:, esl, :].rearrange("d e o -> e d o"))
                wl3 = pool.tile([128, B, NO], f16, tag="wl3")
                nc.gpsimd.dma_start(
                    wl3[:], wl_d[:, esl, :].rearrange("b e o -> e b o"))
                w_t = pool.tile([128, NO], f16, tag="w_t")
                nc.gpsimd.dma_start(w_t[:], w_d[esl, :])
                stdp_t = pool.tile([128, NO], f16, tag="stdp_t")
                nc.gpsimd.dma_start(stdp_t[:], stdp_d[esl, :])
                sgn_t = pool.tile([128, NO], f16, tag="sgn_t")
                nc.gpsimd.dma_start(sgn_t[:], sgn_d[esl, :])

            # A = sgn*W*(1-f), C = sgn*f  (fp16)
            C_t = pool.tile([128, NO], f16, tag="C_t")
            nc.vector.tensor_tensor(C_t[:], sgn_t[:], stdp_t[:], op=op.mult)
            omf = pool.tile([128, NO], f16, tag="omf")
            nc.scalar.activation(
                omf[:], stdp_t[:], mybir.ActivationFunctionType.Copy,
                bias=1.0, scale=-1.0)
            sw = pool.tile([128, NO], f16, tag="sw")
            nc.vector.tensor_tensor(sw[:], sgn_t[:], w_t[:], op=op.mult)
            A_t = pool.tile([128, NO], f16, tag="A_t")
            nc.vector.tensor_tensor(A_t[:], sw[:], omf[:], op=op.mult)

            # Pi = sum_d diag(packed[:,et,d]) @ dmap[d] on the PE
            dstack = pool.tile([128, D, 128], f16, tag="dstack")
            nc.vector.tensor_tensor(
                dstack[:], ident3[:],
                packed16[:, et, :].unsqueeze(-1).broadcast_to((128, D, 128)),
                op=op.mult)
            pi_ps = pspool.tile([128, NO], f32, name=f"pi_ps{et}", tag="pi_ps")
            for d in range(D):
                nc.tensor.matmul(
                    pi_ps[:], dstack[:, d, :], dm3[:, d, :],
                    start=(d == 0), stop=(d == D - 1))
            pi_i16 = pool.tile([128, NO], i16, tag="pi_i16")
            nc.vector.tensor_copy(pi_i16[:], pi_ps[:])

            # masks m01 = (pi >> b) & 1 in i16 (no cast inside bitVec op);
            # one batched cast-copy to f16 on the scalar engine
            m_i16 = pool.tile([128, B, NO], i16, tag="m_i16")
            for b in range(B):
                nc.vector.tensor_scalar(
                    m_i16[:, b, :], pi_i16[:], b, 1,
                    op0=op.logical_shift_right, op1=op.bitwise_and)
            m_f16 = pool.tile([128, B, NO], f16, tag="m_f16")
            nc.scalar.activation(
                m_f16[:], m_i16[:], mybir.ActivationFunctionType.Copy)

            # T[b] = (A + C*Wlong[b]) * m[b], batched over b in 3D APs
            t_all = pool.tile([128, B, NO], f16, tag="t_all")
            nc.vector.tensor_tensor(
                t_all[:], wl3[:],
                C_t[:].unsqueeze(1).broadcast_to((128, B, NO)), op=op.mult)
            nc.vector.tensor_tensor(
                t_all[:], t_all[:],
                A_t[:].unsqueeze(1).broadcast_to((128, B, NO)), op=op.add)
            nc.vector.tensor_tensor(t_all[:], t_all[:], m_f16[:], op=op.mult)

            for b in range(B):
                nc.tensor.matmul(
                    acc[:], ebs[b][:], t_all[:, b, :],
                    start=(et == 0 and b == 0),
                    stop=(et == ET - 1 and b == B - 1))

        out_sb = cpool.tile([B, NO], f32)
        nc.vector.tensor_copy(out_sb[:], acc[:])
        nc.sync.dma_start(out_d[:], out_sb[:])

    nc.compile()
    return nc


def _in_maps(Xd, delaymap, W, Wlong, STDP_frac, signs):
    maps = []
    for c in range(N_CORES):
        h, q = divmod(c, 4)
        e0, o0 = h * NE, q * NO
        es, os_ = slice(e0, e0 + NE), slice(o0, o0 + NO)
        maps.append({
            "dmap": np.ascontiguousarray(delaymap[:, es, os_]),
            "xd": np.ascontiguousarray(Xd[:, :, es]),
            "wl": np.ascontiguousarray(Wlong[:, es, os_]),
            "w": np.ascontiguousarray(W[es, os_]),
            "stdp": np.ascontiguousarray(STDP_frac[es, os_]),
            "sgn": np.ascontiguousarray(signs[es, os_]),
        })
    return maps


def _gather(outs):
    return np.concatenate(
        [outs[q] + outs[q + 4] for q in range(4)], axis=1).astype(np.float32)


def kernel(Xd, delaymap, W, Wlong, STDP_frac, signs):
    global _NC
    from concourse.bass_utils import run_bass_kernel_spmd
    if _NC is None:
        _NC = _build()
    maps = _in_maps(Xd, delaymap, W, Wlong, STDP_frac, signs)
    res = run_bass_kernel_spmd(_NC, maps, list(range(N_CORES)))
    return _gather([r["iout"] for r in res.results])



# revision 1
# speedup vs baseline: 1.1170x; 1.1170x over previous
"""DeltaSynapse (gnn_message_passing) Trainium2 Bass kernel.

Computes I[b,o] = sum_e signs[e,o]*(W[e,o]*(1-f[e,o]) + Wlong[b,e,o]*f[e,o])
                  * Xpre[b,e,o],
with Xpre[b,e,o] = sum_d delaymap[d,e,o]*Xd[d,b,e]  (one-hot delay gather).

Strategy (8 NeuronCores): shard the postsynaptic axis o into 4 quarters of
512 and the presynaptic axis e into 2 halves of 1024; core (h,q) computes
the partial sum over its e-half for its o-quarter. The two e-half partials
are summed on the host (64KB) and the o-quarters concatenated.

On-device per core:
  - Xd is bit-packed once: packed[e,d] = sum_b 2^b * Xd[d,b,e] (PE transpose
    + weighted free-axis reduce).
  - Per e-tile (128 e's x 512 o's):
      Pi[e,o] = sum_d packed[e,d] * dmap[d,e,o]   (one-hot selection => Pi
      holds all 8 per-batch spike masks as an 8-bit integer, exact in fp16)
      m[b] = (uint8(Pi) >> b) & 1                  (per-batch masks)
      T[b] = (A + C*Wlong[b]) * m[b]               (A = sgn*W*(1-f), C=sgn*f)
      I[b,:] += column-sums of T[b] via PE matmul with a one-hot-column
      stationary matrix (lands each batch on its own PSUM partition).
  All bulk tensors are cast f32->f16 by the DMA engines on load; the
  e-reduction accumulates in fp32 PSUM.
"""
import numpy as np
from contextlib import ExitStack

D, B, N = 8, 8, 2048
NO = 512          # o columns per core
NE = 1024         # e rows per core
ET = NE // 128    # e-tiles per core
N_CORES = 8

_NC = None


def _build():
    from concourse import bacc, tile, mybir, masks
    from concourse.alu_op_type import AluOpType as op

    f32 = mybir.dt.float32
    f16 = mybir.dt.float16
    i16 = mybir.dt.int16

    nc = bacc.Bacc("TRN2", target_bir_lowering=False, debug=False)

    dmap_d = nc.dram_tensor("dmap", (D, NE, NO), f32, kind="ExternalInput")
    xd_d = nc.dram_tensor("xd", (D, B, NE), f32, kind="ExternalInput")
    wl_d = nc.dram_tensor("wl", (B, NE, NO), f32, kind="ExternalInput")
    w_d = nc.dram_tensor("w", (NE, NO), f32, kind="ExternalInput")
    stdp_d = nc.dram_tensor("stdp", (NE, NO), f32, kind="ExternalInput")
    sgn_d = nc.dram_tensor("sgn", (NE, NO), f32, kind="ExternalInput")
    out_d = nc.dram_tensor("iout", (B, NO), f32, kind="ExternalOutput")

    with tile.TileContext(nc) as tc, ExitStack() as ctx:
        cpool = ctx.enter_context(tc.tile_pool(name="const", bufs=1))
        pool = ctx.enter_context(tc.tile_pool(name="work", bufs=2))
        pspool = ctx.enter_context(tc.tile_pool(name="pst", bufs=2, space="PSUM"))
        accpool = ctx.enter_context(tc.tile_pool(name="acc", bufs=1, space="PSUM"))

        # ---- first two tiles' loads first: no slot waits exist yet, so
        # these issue immediately and DMA runs during the constant setup
        pre = {}
        for et in range(2):
            esl = slice(et * 128, (et + 1) * 128)
            dm3 = pool.tile([128, D, NO], f16, name=f"dm3_{et}", tag="dm3")
            nc.gpsimd.dma_start(
                dm3[:], dmap_d[:, esl, :].rearrange("d e o -> e d o"))
            wl3 = pool.tile([128, B, NO], f16, name=f"wl3_{et}", tag="wl3")
            nc.gpsimd.dma_start(
                wl3[:], wl_d[:, esl, :].rearrange("b e o -> e b o"))
            w_t = pool.tile([128, NO], f16, name=f"w_{et}", tag="w_t")
            nc.gpsimd.dma_start(w_t[:], w_d[esl, :])
            stdp_t = pool.tile([128, NO], f16, name=f"st_{et}", tag="stdp_t")
            nc.gpsimd.dma_start(stdp_t[:], stdp_d[esl, :])
            sgn_t = pool.tile([128, NO], f16, name=f"sg_{et}", tag="sgn_t")
            nc.gpsimd.dma_start(sgn_t[:], sgn_d[esl, :])
            pre[et] = (dm3, wl3, w_t, stdp_t, sgn_t)

        # ---- constants -------------------------------------------------
        ident = cpool.tile([D * B, D * B], f32)
        masks.make_identity(nc, ident[:])
        ebs = []
        for b in range(B):
            ebt = cpool.tile([128, B], f16, name=f"eb{b}")
            nc.vector.memset(ebt[:], 0.0)
            nc.vector.memset(ebt[:, b:b + 1], 1.0)
            ebs.append(ebt)
        pw = cpool.tile([128, D, B], f32)
        for b in range(B):
            nc.vector.memset(pw[:, :, b], float(1 << b))
        # stack of 8 identity matrices (f16) for building diag(packed[d])
        ident3 = cpool.tile([128, D, 128], f16)
        for d in range(D):
            masks.make_identity(nc, ident3[:, d, :])

        # ---- pack Xd: packed[e, et, d] = sum_b 2^b * Xd[d, b, e] -------
        xd_nat = cpool.tile([D * B, NE], f32)
        nc.sync.dma_start(xd_nat[:], xd_d[:].flatten_outer_dims())
        packed = cpool.tile([128, ET, D], f32)
        for c in range(ET):
            xdt_ps = pspool.tile([128, D * B], f32, name=f"xdt{c}", tag="xdt")
            nc.tensor.matmul(
                xdt_ps[:], xd_nat[:, c * 128:(c + 1) * 128], ident[:],
                is_transpose=True)
            xw = pool.tile([128, D, B], f32, name=f"xw{c}", tag="xw")
            nc.vector.tensor_tensor(
                xw[:], xdt_ps[:].rearrange("e (d b) -> e d b", d=D), pw[:],
                op=op.mult)
            nc.vector.tensor_reduce(
                packed[:, c, :], xw[:], axis=mybir.AxisListType.X, op=op.add)
        packed16 = cpool.tile([128, ET, D], f16)
        nc.vector.tensor_copy(packed16[:], packed[:])

        acc = accpool.tile([B, NO], f32)

        # ---- main loop over e-tiles ------------------------------------
        for et in range(ET):
            esl = slice(et * 128, (et + 1) * 128)

            if et in pre:
                dm3, wl3, w_t, stdp_t, sgn_t = pre[et]
            else:
                dm3 = pool.tile([128, D, NO], f16, tag="dm3")
                nc.gpsimd.dma_start(
                    dm3[:], dmap_d[:, esl, :].rearrange("d e o -> e d o"))
                wl3 = pool.tile([128, B, NO], f16, tag="wl3")
                nc.gpsimd.dma_start(
                    wl3[:], wl_d[:, esl, :].rearrange("b e o -> e b o"))
                w_t = pool.tile([128, NO], f16, tag="w_t")
                nc.gpsimd.dma_start(w_t[:], w_d[esl, :])
                stdp_t = pool.tile([128, NO], f16, tag="stdp_t")
                nc.gpsimd.dma_start(stdp_t[:], stdp_d[esl, :])
                sgn_t = pool.tile([128, NO], f16, tag="sgn_t")
                nc.gpsimd.dma_start(sgn_t[:], sgn_d[esl, :])

            # A = sgn*W*(1-f), C = sgn*f  (fp16)
            C_t = pool.tile([128, NO], f16, tag="C_t")
            nc.vector.tensor_tensor(C_t[:], sgn_t[:], stdp_t[:], op=op.mult)
            omf = pool.tile([128, NO], f16, tag="omf")
            nc.scalar.activation(
                omf[:], stdp_t[:], mybir.ActivationFunctionType.Copy,
                bias=1.0, scale=-1.0)
            sw = pool.tile([128, NO], f16, tag="sw")
            nc.vector.tensor_tensor(sw[:], sgn_t[:], w_t[:], op=op.mult)
            A_t = pool.tile([128, NO], f16, tag="A_t")
            nc.vector.tensor_tensor(A_t[:], sw[:], omf[:], op=op.mult)

            # Pi = sum_d diag(packed[:,et,d]) @ dmap[d] on the PE
            dstack = pool.tile([128, D, 128], f16, tag="dstack")
            nc.vector.tensor_tensor(
                dstack[:], ident3[:],
                packed16[:, et, :].unsqueeze(-1).broadcast_to((128, D, 128)),
                op=op.mult)
            pi_ps = pspool.tile([128, NO], f32, name=f"pi_ps{et}", tag="pi_ps")
            for d in range(D):
                nc.tensor.matmul(
                    pi_ps[:], dstack[:, d, :], dm3[:, d, :],
                    start=(d == 0), stop=(d == D - 1))
            pi_i16 = pool.tile([128, NO], i16, tag="pi_i16")
            nc.vector.tensor_copy(pi_i16[:], pi_ps[:])

            # masks m01 = (pi >> b) & 1 in i16 (no cast inside bitVec op);
            # one batched cast-copy to f16 on the scalar engine
            m_i16 = pool.tile([128, B, NO], i16, tag="m_i16")
            for b in range(B):
                nc.vector.tensor_scalar(
                    m_i16[:, b, :], pi_i16[:], b, 1,
                    op0=op.logical_shift_right, op1=op.bitwise_and)
            m_f16 = pool.tile([128, B, NO], f16, tag="m_f16")
            nc.scalar.activation(
                m_f16[:], m_i16[:], mybir.ActivationFunctionType.Copy)

            # T[b] = (A + C*Wlong[b]) * m[b], batched over b in 3D APs
            t_all = pool.tile([128, B, NO], f16, tag="t_all")
            nc.vector.tensor_tensor(
                t_all[:], wl3[:],
                C_t[:].unsqueeze(1).broadcast_to((128, B, NO)), op=op.mult)
            nc.vector.tensor_tensor(
                t_all[:], t_all[:],
                A_t[:].unsqueeze(1).broadcast_to((128, B, NO)), op=op.add)
            nc.vector.tensor_tensor(t_all[:], t_all[:], m_f16[:], op=op.mult)

            for b in range(B):
                nc.tensor.matmul(
                    acc[:], ebs[b][:], t_all[:, b, :],
                    start=(et == 0 and b == 0),
                    stop=(et == ET - 1 and b == B - 1))

        out_sb = cpool.tile([B, NO], f32)
        nc.vector.tensor_copy(out_sb[:], acc[:])
        nc.sync.dma_start(out_d[:], out_sb[:])

    nc.compile()
    return nc


def _in_maps(Xd, delaymap, W, Wlong, STDP_frac, signs):
    maps = []
    for c in range(N_CORES):
        h, q = divmod(c, 4)
        e0, o0 = h * NE, q * NO
        es, os_ = slice(e0, e0 + NE), slice(o0, o0 + NO)
        maps.append({
            "dmap": np.ascontiguousarray(delaymap[:, es, os_]),
            "xd": np.ascontiguousarray(Xd[:, :, es]),
            "wl": np.ascontiguousarray(Wlong[:, es, os_]),
            "w": np.ascontiguousarray(W[es, os_]),
            "stdp": np.ascontiguousarray(STDP_frac[es, os_]),
            "sgn": np.ascontiguousarray(signs[es, os_]),
        })
    return maps


def _gather(outs):
    return np.concatenate(
        [outs[q] + outs[q + 4] for q in range(4)], axis=1).astype(np.float32)


def kernel(Xd, delaymap, W, Wlong, STDP_frac, signs):
    global _NC
    from concourse.bass_utils import run_bass_kernel_spmd
    if _NC is None:
        _NC = _build()
    maps = _in_maps(Xd, delaymap, W, Wlong, STDP_frac, signs)
    res = run_bass_kernel_spmd(_NC, maps, list(range(N_CORES)))
    return _gather([r["iout"] for r in res.results])

